# revision 3
# baseline (speedup 1.0000x reference)
"""FlowNetC correlation kernel for Trainium2 (Bass/Tile), 8-core data-parallel.

Problem: in1, in2: [B=8, C=256, H=96, W=128] fp32
  out[b, o, y, x] = (1/C) * sum_c in1[b,c,y,x] * in2pad[b,c,y+dy,x+dx]
  dy = (o//21 - 10)*2, dx = (o%21 - 10)*2   (441 displacement planes)

Strategy (v2):
  * Data-parallel over batch: one sample per NeuronCore (8 cores).
  * Displacements are even in both axes -> split both images into 4 parity
    classes (y%2, x%2); each class is an independent stride-1 correlation of
    [C, 48, 64] with +-10 neighborhood.
  * bf16 matmuls (PE runs bf16 at 1 cycle/row vs fp32's 4): inputs are cast
    to bf16 on the host (1/C scale folded into in1 first; the 2e-2 rel-err
    budget dwarfs bf16's ~1e-3).
  * in2 is shipped UNPADDED ([C, 48, 64] per class); edge tiles simply use a
    clipped in2 window. Clipped-out displacements correspond to zero-padding
    in the reference and are zero-filled on the host. This cuts both matmul
    work and dump bytes to 74% of the padded-window version.
  * Per class, 2D-tile Gram on the PE array: stationary lhsT = 16x8 = 128
    in1 pixels [C-chunk(128) x M=128]; moving rhs = the clipped in2 window
    around the tile (rows x cols <= 36x28), contracted over C in 2 chunks,
    split row-wise into two <=504-col matmuls (one PSUM bank each).
  * PSUM -> SBUF copy casts fp32 -> bf16; one DMA per tile dumps the packed
    window to DRAM (4 output tensors, grouped by window size: zero waste).
  * All 8 input chunks (4 classes x 2 C-chunks, a+b) are preloaded into SBUF
    up front (bf16 makes this ~96 KiB/partition total) so the PE never waits
    on input DMA after the first chunk.
  * Band extraction (shear) and layout permutation run on the host
    (numpy as_strided), costing no device time.
"""

import os
import numpy as np
from contextlib import ExitStack

import concourse.bass as bass
import concourse.bacc as bacc
import concourse.tile as tile
import concourse.mybir as mybir
from concourse import bass2jax

# ---- problem geometry (hardcoded) ----
B, C, H, W = 8, 256, 96, 128
R = 10                     # class-space displacement radius
GW = 2 * R + 1             # 21
NOUT = GW * GW             # 441
HC, WC = H // 2, W // 2    # 48, 64  class image dims
H1T, W1T = 16, 8           # in1 tile -> M = 128 pixels
NTY, NTX = HC // H1T, WC // W1T      # 3 x 8 = 24 tiles per class
NTILE = NTY * NTX
H2T, W2T = H1T + 2 * R, W1T + 2 * R  # 36, 28 full (interior) window
KCH = C // 128             # 2 contraction chunks

MM_DT = {
    "fp32": mybir.dt.float32,
    "fp32r": mybir.dt.float32r,
    "bf16": mybir.dt.bfloat16,
}[os.environ.get("CORR_DT", "bf16")]
OUT_DT = {
    "fp32": mybir.dt.float32,
    "bf16": mybir.dt.bfloat16,
}[os.environ.get("CORR_OUT_DT", "bf16")]


def _tile_geom(ty, tx):
    """Clipped in2 window for tile (ty, tx): rows [r0,r1) x cols [c0,c1)."""
    ya, xa = ty * H1T, tx * W1T
    r0, r1 = max(0, ya - R), min(HC, ya + H1T + R)
    c0, c1 = max(0, xa - R), min(WC, xa + W1T + R)
    return r0, r1, c0, c1


# tiles grouped by (rows, cols) so each group's dump tensor is fully packed
_GROUPS: dict[tuple[int, int], list[tuple[int, int]]] = {}
for _ty in range(NTY):
    for _tx in range(NTX):
        _r0, _r1, _c0, _c1 = _tile_geom(_ty, _tx)
        _GROUPS.setdefault((_r1 - _r0, _c1 - _c0), []).append((_ty, _tx))
_GKEYS = sorted(_GROUPS.keys())          # deterministic tensor order
_GIDX = {  # (ty,tx) -> (group key, index within group)
    t: (k, i) for k in _GKEYS for i, t in enumerate(_GROUPS[k])
}

_CACHE = {}


def _build_nc(mm_dt, out_dt):
    nc = bacc.Bacc(
        trn_type="TRN2",
        target_bir_lowering=False,
        debug=False,
        num_devices=8,
    )
    # a: in1 pre-tiled on host so each tile's 128 pixels are contiguous
    #    (stationary matmul operand must have a single free dim).
    a_h = nc.dram_tensor("a", [4, C, NTILE, 128], mm_dt, kind="ExternalInput")
    # b: in2 parity classes, unpadded
    b_h = nc.dram_tensor("b", [4, C, HC, WC], mm_dt, kind="ExternalInput")
    o_h = {
        (rows, cols): nc.dram_tensor(
            f"o{rows}x{cols}", [4, len(_GROUPS[(rows, cols)]), 128, rows * cols],
            out_dt, kind="ExternalOutput")
        for (rows, cols) in _GKEYS
    }
    a_ap, b_ap = a_h.ap(), b_h.ap()
    o_ap = {k: v.ap() for k, v in o_h.items()}

    with tile.TileContext(nc) as tc, ExitStack() as ctx:
        a_pool = ctx.enter_context(tc.tile_pool(name="a", bufs=4 * KCH))
        b_pool = ctx.enter_context(tc.tile_pool(name="b", bufs=4 * KCH))
        s_pool = ctx.enter_context(tc.tile_pool(name="stage", bufs=6))
        p_pool = ctx.enter_context(tc.tile_pool(name="psum", bufs=3, space="PSUM"))
        pd_pool = ctx.enter_context(tc.tile_pool(name="psd", bufs=1, space="PSUM"))
        ps_dummy = pd_pool.tile([128, 8], mybir.dt.float32)

        # preload everything: 4 classes x 2 C-chunks of a and b
        a_t, b_t = {}, {}
        for cls in range(4):
            for kc in range(KCH):
                at = a_pool.tile([128, NTILE, 128], mm_dt, tag="a")
                nc.scalar.dma_start(out=at[:], in_=a_ap[cls, kc * 128:(kc + 1) * 128])
                a_t[cls, kc] = at
                bt = b_pool.tile([128, HC, WC], mm_dt, tag="b")
                nc.scalar.dma_start(out=bt[:], in_=b_ap[cls, kc * 128:(kc + 1) * 128])
                b_t[cls, kc] = bt
        # single-wait "touchers": first PE consumer of each loaded tile
        # carries exactly one DMA wait (fused LDW+MM supports only one).
        for cls in range(4):
            for kc in range(KCH):
                nc.tensor.matmul(ps_dummy[0:1, 0:1], a_t[cls, kc][:, 0, 0:1],
                                 a_t[cls, kc][:, 0, 0:1], start=True, stop=True)
                nc.tensor.matmul(ps_dummy[0:1, 0:1], b_t[cls, kc][:, 0, 0:1],
                                 b_t[cls, kc][:, 0, 0:1], start=True, stop=True)

        for cls in range(4):
            for ty in range(NTY):
                for tx in range(NTX):
                    r0, r1, c0, c1 = _tile_geom(ty, tx)
                    rows, cols = r1 - r0, c1 - c0
                    h1 = rows // 2
                    n1, n2 = h1 * cols, (rows - h1) * cols
                    nt = rows * cols
                    t = ty * NTX + tx
                    gkey, gi = _GIDX[(ty, tx)]
                    ps = p_pool.tile([128, 1024], mybir.dt.float32)
                    for kc in range(KCH):
                        lhsT = a_t[cls, kc][:, t, :]
                        rhs_top = b_t[cls, kc][:, r0:r0 + h1, c0:c1]
                        rhs_bot = b_t[cls, kc][:, r0 + h1:r1, c0:c1]
                        nc.tensor.matmul(ps[:, 0:n1], lhsT, rhs_top,
                                         start=(kc == 0), stop=(kc == KCH - 1))
                        nc.tensor.matmul(ps[:, 512:512 + n2], lhsT, rhs_bot,
                                         start=(kc == 0), stop=(kc == KCH - 1))
                    sb = s_pool.tile([128, H2T * W2T], out_dt, tag="sb")
                    nc.vector.tensor_copy(sb[:, 0:n1], ps[:, 0:n1])
                    nc.scalar.copy(sb[:, n1:nt], ps[:, 512:512 + n2])
                    nc.sync.dma_start(out=o_ap[gkey][cls, gi], in_=sb[:, 0:nt])
    nc.compile()
    return nc


_MM_NP = {
    mybir.dt.float32: np.float32,
    mybir.dt.float32r: np.float32,
}.get(MM_DT)
if _MM_NP is None:
    import ml_dtypes
    _MM_NP = ml_dtypes.bfloat16


def _host_prep(input1, input2):
    """Build per-core input arrays: parity classes, fold in 1/C, cast."""
    x1 = (input1.astype(np.float32) * np.float32(1.0 / C))
    # [B, C, H, W] -> [B, 4, C, HC, WC] with class = (y%2)*2 + (x%2)
    x1 = x1.reshape(B, C, HC, 2, WC, 2).transpose(0, 3, 5, 1, 2, 4)
    x1 = np.ascontiguousarray(x1).reshape(B, 4, C, HC, WC)
    # pre-tile: [.., HC, WC] -> [.., NTILE, 128] with pixel (u, v) contiguous
    x1 = x1.reshape(B, 4, C, NTY, H1T, NTX, W1T).transpose(0, 1, 2, 3, 5, 4, 6)
    x1 = np.ascontiguousarray(x1).reshape(B, 4, C, NTILE, 128).astype(_MM_NP)
    x2 = input2.astype(np.float32)
    x2 = x2.reshape(B, C, HC, 2, WC, 2).transpose(0, 3, 5, 1, 2, 4)
    x2 = np.ascontiguousarray(x2).reshape(B, 4, C, HC, WC).astype(_MM_NP)
    return x1, x2


def _host_extract(res):
    """res: {name: [4, cnt, 128, rows*cols]} dumps for one sample ->
    out [441, 96, 128]."""
    # paste each tile's clipped window into a zero canvas [4,24,128,36,28]
    # positioned so canvas[yy, xx] = padded-window[(u+i2), (v+j2)]
    canvas = np.zeros((4, NTILE, 128, H2T, W2T), dtype=np.float32)
    for (rows, cols), tlist in _GROUPS.items():
        name = f"o{rows}x{cols}"
        band = np.asarray(res[name], dtype=np.float32).reshape(
            4, len(tlist), 128, rows, cols)
        for i, (ty, tx) in enumerate(tlist):
            r0, _, c0, _ = _tile_geom(ty, tx)
            oy = ty * H1T - R - r0   # canvas yy = band row - oy
            ox = tx * W1T - R - c0
            t = ty * NTX + tx
            canvas[:, t, :, -oy: -oy + rows, -ox: -ox + cols] = band[:, i]
    r = canvas.reshape(4, NTY, NTX, H1T, W1T, H2T, W2T)
    se = r.strides
    # V[cls, ty, tx, u, v, i2, j2] = r[cls, ty, tx, u, v, u+i2, v+j2]
    V = np.lib.stride_tricks.as_strided(
        r,
        shape=(4, NTY, NTX, H1T, W1T, GW, GW),
        strides=(se[0], se[1], se[2], se[3] + se[5], se[4] + se[6], se[5], se[6]),
    )
    # cls = (py, px); out[(i2,j2), (ty,u,py), (tx,v,px)]
    V = V.reshape(2, 2, NTY, NTX, H1T, W1T, GW, GW)
    out = V.transpose(6, 7, 2, 4, 0, 3, 5, 1)  # i2, j2, ty, u, py, tx, v, px
    return np.ascontiguousarray(out).reshape(NOUT, H, W)


def _make_runner(nc, n_cores=B):
    """Cached jitted SPMD runner (mirrors bass2jax.run_bass_via_pjrt, but
    reusable across calls so the NEFF compiles once per process)."""
    import jax
    from jax.sharding import Mesh, PartitionSpec
    from jax.experimental.shard_map import shard_map

    bass2jax.install_neuronx_cc_hook()

    partition_name = (nc.partition_id_tensor.name
                      if nc.partition_id_tensor else None)
    in_names, out_names, out_avals, zero_outs = [], [], [], []
    for alloc in nc.m.functions[0].allocations:
        if not isinstance(alloc, mybir.MemoryLocationSet):
            continue
        name = alloc.memorylocations[0].name
        if alloc.kind == "ExternalInput":
            if name != partition_name:
                in_names.append(name)
        elif alloc.kind == "ExternalOutput":
            out_names.append(name)
            shape = tuple(alloc.tensor_shape)
            dtype = mybir.dt.np(alloc.dtype)
            out_avals.append(jax.core.ShapedArray(shape, dtype))
            zero_outs.append(np.zeros(shape, dtype))
    n_params = len(in_names)
    n_outs = len(out_avals)
    all_names = in_names + out_names
    if partition_name is not None:
        all_names = all_names + [partition_name]
    donate = tuple(range(n_params, n_params + n_outs))

    def _body(*args):
        operands = list(args)
        if partition_name is not None:
            operands.append(bass2jax.partition_id_tensor())
        outs = bass2jax._bass_exec_p.bind(
            *operands,
            out_avals=tuple(out_avals),
            in_names=tuple(all_names),
            out_names=tuple(out_names),
            lowering_input_output_aliases=(),
            sim_require_finite=True,
            sim_require_nnan=True,
            nc=nc,
        )
        return tuple(outs)

    devices = jax.devices()[:n_cores]
    mesh = Mesh(np.asarray(devices), ("core",))
    in_specs = (PartitionSpec("core"),) * (n_params + n_outs)
    out_specs = (PartitionSpec("core"),) * n_outs
    sharded = jax.jit(
        shard_map(_body, mesh=mesh, in_specs=in_specs, out_specs=out_specs,
                  check_rep=False),
        donate_argnums=donate, keep_unused=True,
    )
    return {
        "fn": sharded, "in_names": in_names, "out_names": out_names,
        "out_avals": out_avals, "zero_outs": zero_outs, "mesh": mesh,
        "n_cores": n_cores,
    }


def _run_spmd(runner, in_maps):
    """Execute; returns list per core of {name: np.ndarray}."""
    import jax
    n_cores = runner["n_cores"]
    concat_in = [
        np.concatenate([np.asarray(in_maps[c][name]) for c in range(n_cores)], axis=0)
        for name in runner["in_names"]
    ]
    concat_zeros = [
        np.zeros((n_cores * z.shape[0], *z.shape[1:]), z.dtype)
        for z in runner["zero_outs"]
    ]
    out_arrs = runner["fn"](*concat_in, *concat_zeros)
    out_arrs = jax.block_until_ready(out_arrs)
    results = [
        {
            name: np.asarray(out_arrs[i]).reshape(n_cores, *runner["out_avals"][i].shape)[c]
            for i, name in enumerate(runner["out_names"])
        }
        for c in range(n_cores)
    ]
    return results


def time_exec(runner, in_maps, iters=10):
    """Device-execute wall time with inputs pre-transferred (seconds, min)."""
    import time as _time
    import jax
    from jax.sharding import NamedSharding, PartitionSpec
    n_cores = runner["n_cores"]
    sh = NamedSharding(runner["mesh"], PartitionSpec("core"))
    concat_in = [
        jax.device_put(
            np.concatenate([np.asarray(in_maps[c][name]) for c in range(n_cores)],
                           axis=0), sh)
        for name in runner["in_names"]
    ]
    best = None
    for _ in range(iters):
        zeros = [
            jax.device_put(
                np.zeros((n_cores * z.shape[0], *z.shape[1:]), z.dtype), sh)
            for z in runner["zero_outs"]
        ]
        jax.block_until_ready(zeros)
        jax.block_until_ready(concat_in)
        t0 = _time.perf_counter()
        outs = runner["fn"](*concat_in, *zeros)
        jax.block_until_ready(outs)
        dt = _time.perf_counter() - t0
        best = dt if best is None else min(best, dt)
    return best


def get_runner():
    if "runner" not in _CACHE:
        _CACHE["nc"] = _build_nc(MM_DT, OUT_DT)
        _CACHE["runner"] = _make_runner(_CACHE["nc"])
    return _CACHE["runner"]


def kernel(input1, input2):
    assert input1.shape == (B, C, H, W) and input2.shape == (B, C, H, W)
    x1, x2 = _host_prep(np.asarray(input1), np.asarray(input2))
    runner = get_runner()
    in_maps = [{"a": x1[b], "b": x2[b]} for b in range(B)]
    results = _run_spmd(runner, in_maps)
    out = np.empty((B, NOUT, H, W), dtype=np.float32)
    for b in range(B):
        out[b] = _host_extract(results[b])
    return out
